# revision 20
# baseline (speedup 1.0000x reference)
"""Trainium2 Bass kernel for nn_EDMLoss (VQ codebook loss).

Strategy (8 NeuronCores, data-parallel over batch B=8, one batch row per core):
  - L1 nearest-codeword search: per codeword k, |H - M_k| in bf16, split
    2:1 between DVE (tensor_scalar subtract + uint32 sign-mask AND) and
    ScalarE (fused activation Abs with per-partition bias = -M_k); the
    D-reduction runs on the PE as bf16 matmuls with negated selector
    weights ([128,32], -1 in column k%32) accumulating 32 codewords per
    PSUM tile at the four tile_position column groups -> PSUM = -d[k, t].
  - PE transpose-mode -> -d[t, k]; first-match argmin via the DVE Max8
    (nc.vector.max) + max_index units on the negated distances.
  - Loss terms assembled exactly in fp32: sum(H-Z)^2 = sum H^2 - 2*G[t,k*]
    + ||M_k*||^2, with G = H^T M from an exact fp32 matmul and the
    per-token gathers done by gpsimd indirect_copy (16-wide group gather)
    + a diagonal-mask reduction.
  - Recon/disc losses + adaptive-weight grad partials via fp32 matmuls.
  - Tiny per-core partials ([128,40] + [33,256] per core) are summed on
    the host in float64 and combined into the scalar loss.
"""

import numpy as np

B, T, C, F, D, K = 8, 1024, 32, 256, 128, 512
ALPHA, GAMMA = 1.0, 1e-6
NCORES = 8
NT = T // 128          # 8 token chunks of 128
NKB = K // 128         # 4 codeword blocks of 128
ENG_PAT = ('D', 'D', 'A')  # abs-engine cycle: DVE, DVE, ScalarE

_NC_CACHE = {}


def _build_nc(reps=1):
    import concourse.bacc as bacc
    import concourse.tile as tile
    from concourse import mybir
    from concourse.masks import make_identity

    f32 = mybir.dt.float32
    bf16 = mybir.dt.bfloat16
    u32 = mybir.dt.uint32
    Alu = mybir.AluOpType
    Act = mybir.ActivationFunctionType

    nc = bacc.Bacc("TRN2", target_bir_lowering=False)
    H_d = nc.dram_tensor("H", [D, T], f32, kind="ExternalInput")
    M_d = nc.dram_tensor("M", [D, K], f32, kind="ExternalInput")
    X_d = nc.dram_tensor("X", [T, C], f32, kind="ExternalInput")
    Hd_d = nc.dram_tensor("Hd", [T, F], f32, kind="ExternalInput")
    W_d = nc.dram_tensor("W", [C, F], f32, kind="ExternalInput")
    wd_d = nc.dram_tensor("wd", [1, C], f32, kind="ExternalInput")
    acc_d = nc.dram_tensor("acc", [128, 40], f32, kind="ExternalOutput")
    grs_d = nc.dram_tensor("grs", [C + 1, F], f32, kind="ExternalOutput")

    with tile.TileContext(nc) as tc:
        with (
            tc.tile_pool(name="consts", bufs=1) as consts,
            tc.tile_pool(name="pabs", bufs=12) as pabs,
            tc.tile_pool(name="pdsb", bufs=2) as pdsb,
            tc.tile_pool(name="psml", bufs=8) as psml,
            tc.tile_pool(name="pp_d", bufs=3, space="PSUM") as pp_d,
            tc.tile_pool(name="pp_tr", bufs=2, space="PSUM") as pp_tr,
            tc.tile_pool(name="pp_g", bufs=2, space="PSUM") as pp_g,
        ):
            # ---------- input DMAs ----------
            H_sb = consts.tile([D, T], f32)
            M_sb = consts.tile([D, K], f32)
            nc.sync.dma_start(out=H_sb, in_=H_d[:, :])
            nc.sync.dma_start(out=M_sb, in_=M_d[:, :])
            X_sb = consts.tile([128, NT, C], f32)
            nc.sync.dma_start(
                out=X_sb, in_=X_d.rearrange("(n p) c -> p n c", p=128))
            Hd_sb = consts.tile([128, NT, F], f32)
            nc.sync.dma_start(
                out=Hd_sb, in_=Hd_d.rearrange("(n p) f -> p n f", p=128))
            W_sb = consts.tile([C, F], f32)
            nc.sync.dma_start(out=W_sb, in_=W_d[:, :])
            wd_sb = consts.tile([1, C], f32)
            nc.sync.dma_start(out=wd_sb, in_=wd_d[:, :])

            # ---------- constants ----------
            H_bf = consts.tile([D, T], bf16)
            nc.vector.tensor_copy(out=H_bf, in_=H_sb)
            M_neg = consts.tile([D, K], f32)
            nc.vector.tensor_scalar(
                out=M_neg, in0=M_sb, scalar1=-1.0, scalar2=None, op0=Alu.mult)

            sel = consts.tile([128, 64], bf16)   # col 32 = -1 -> PSUM = -d
            nc.vector.memset(sel, 0.0)
            nc.vector.memset(sel[:, 32:33], -1.0)
            ident = consts.tile([128, 128], f32)
            make_identity(nc, ident)

            # diag16[p, j] = (j == p % 16), for indirect_copy extraction
            iota_i = consts.tile([128, 16], mybir.dt.int32)
            nc.gpsimd.iota(iota_i, pattern=[[1, 16]], base=0,
                           channel_multiplier=-1)
            iota_m = consts.tile([128, 16], mybir.dt.int32)
            nc.vector.tensor_scalar(
                out=iota_m, in0=iota_i, scalar1=15, scalar2=None,
                op0=Alu.bitwise_and)
            diag16 = consts.tile([128, 16], f32)
            nc.vector.tensor_scalar(
                out=diag16, in0=iota_m, scalar1=0, scalar2=None,
                op0=Alu.is_equal)

            ones_col = consts.tile([128, 1], f32)
            nc.vector.memset(ones_col, 1.0)
            zbias = consts.tile([128, 1], f32)
            nc.vector.memset(zbias, 0.0)
            ones_row = consts.tile([1, 128], f32)
            nc.vector.memset(ones_row, 1.0)

            acc_sb = consts.tile([128, 40], f32)
            nc.vector.memset(acc_sb, 0.0)

            # ---------- main loop: distances d[k, t] ----------
            dT_all = consts.tile([128, NT, K], f32)
            for kb in [kb for _ in range(reps) for kb in range(NKB)]:
                dA = pp_d.tile([128, 512], f32, tag="dps")
                dB = pp_d.tile([128, 512], f32, tag="dps")
                for r in range(32):
                    for j in range(4):
                        k = kb * 128 + 32 * j + r
                        ABS = pabs.tile([D, T], bf16, tag="abs")
                        eng = ENG_PAT[k % len(ENG_PAT)]
                        if eng == 'A':
                            nc.scalar.activation(
                                out=ABS, in_=H_bf, func=Act.Abs,
                                bias=M_neg[:, k:k + 1], scale=1.0)
                        else:
                            Y = pabs.tile([D, T], bf16, tag="yab")
                            nc.vector.tensor_scalar(
                                out=Y, in0=H_bf, scalar1=M_sb[:, k:k + 1],
                                scalar2=None, op0=Alu.subtract)
                            nc.vector.tensor_scalar(
                                out=ABS.bitcast(u32), in0=Y.bitcast(u32),
                                scalar1=0x7FFF7FFF, scalar2=None,
                                op0=Alu.bitwise_and)
                        nc.tensor.matmul(
                            out=dA[32 * j:32 * j + 32, :],
                            lhsT=sel[:, 32 - r:64 - r], rhs=ABS[:, 0:512],
                            start=(r == 0), stop=(r == 31),
                            tile_position=(0, 32 * j),
                            skip_group_check=True)
                        nc.tensor.matmul(
                            out=dB[32 * j:32 * j + 32, :],
                            lhsT=sel[:, 32 - r:64 - r], rhs=ABS[:, 512:1024],
                            start=(r == 0), stop=(r == 31),
                            tile_position=(0, 32 * j),
                            skip_group_check=True)
                d_sb = pdsb.tile([128, T], f32, tag="dsb")
                nc.scalar.copy(out=d_sb[:, 0:512], in_=dA)
                nc.scalar.copy(out=d_sb[:, 512:1024], in_=dB)
                for c in range(NT):
                    trp = pp_tr.tile([128, 128], f32, tag="tr")
                    nc.tensor.transpose(
                        out=trp, in_=d_sb[:, c * 128:(c + 1) * 128],
                        identity=ident)
                    nc.scalar.copy(
                        out=dT_all[:, c, kb * 128:(kb + 1) * 128], in_=trp)

            # msq[k] = sum_d M[d,k]^2, broadcast to [128, K]
            SQM = consts.tile([D, K], f32)
            nc.scalar.activation(out=SQM, in_=M_sb, func=Act.Square,
                                 bias=zbias, scale=1.0)
            msqr_ps = pp_g.tile([1, K], f32, tag="gp")
            nc.tensor.matmul(out=msqr_ps, lhsT=ones_col, rhs=SQM,
                             start=True, stop=True)
            msq_row = consts.tile([1, K], f32)
            nc.scalar.copy(out=msq_row, in_=msqr_ps)
            msqbc_ps = pp_g.tile([128, K], f32, tag="gp")
            nc.tensor.matmul(out=msqbc_ps, lhsT=ones_row, rhs=msq_row,
                             start=True, stop=True)
            msq_bc = consts.tile([128, K], f32)
            nc.scalar.copy(out=msq_bc, in_=msqbc_ps)

            # w_d broadcast to [128, C]
            wdbc_ps = pp_g.tile([128, C], f32, tag="gp")
            nc.tensor.matmul(out=wdbc_ps, lhsT=ones_row, rhs=wd_sb,
                             start=True, stop=True)
            wd_bc = consts.tile([128, C], f32)
            nc.scalar.copy(out=wd_bc, in_=wdbc_ps)

            # ---------- G = H^T M (exact fp32), per token chunk ----------
            G_sb = consts.tile([128, NT, K], f32)
            for c in range(NT):
                g_ps = pp_g.tile([128, K], f32, tag="gp")
                nc.tensor.matmul(out=g_ps,
                                 lhsT=H_sb[:, c * 128:(c + 1) * 128],
                                 rhs=M_sb, start=True, stop=True)
                nc.scalar.copy(out=G_sb[:, c, :], in_=g_ps)

            # ---------- sum H^2 (exact fp32 accumulate) ----------
            hsq_scr = pdsb.tile([128, T], f32, tag="dsb")
            nc.vector.scalar_tensor_tensor(
                out=hsq_scr, in0=H_sb, scalar=0.0, in1=H_sb,
                op0=Alu.bypass, op1=Alu.mult, accum_out=acc_sb[:, 16:17])

            # ---------- part 2: recon/disc losses + grad partials ----------
            WT_sb = consts.tile([128, 2, C], f32)
            for fh in range(2):
                wt_ps = pp_tr.tile([128, 128], f32, tag="tr")
                nc.tensor.transpose(
                    out=wt_ps[:, 0:C],
                    in_=W_sb[:, fh * 128:(fh + 1) * 128],
                    identity=ident[0:C, 0:C])
                nc.scalar.copy(out=WT_sb[:, fh, :], in_=wt_ps[:, 0:C])

            HdT_sb = consts.tile([128, 2, T], f32)
            for c in range(NT):
                for fh in range(2):
                    ht_ps = pp_tr.tile([128, 128], f32, tag="tr")
                    nc.tensor.transpose(
                        out=ht_ps,
                        in_=Hd_sb[:, c, fh * 128:(fh + 1) * 128],
                        identity=ident)
                    nc.scalar.copy(
                        out=HdT_sb[:, fh, c * 128:(c + 1) * 128], in_=ht_ps)

            E_ext = consts.tile([128, NT, C + 1], f32)
            nc.vector.memset(E_ext[:, :, C:C + 1], 1.0)
            grs_ps = pp_g.tile([C + 1, F], f32, tag="gp")
            for c in range(NT):
                xh_ps = pp_g.tile([128, C], f32, tag="gp")
                for fh in range(2):
                    nc.tensor.matmul(
                        out=xh_ps,
                        lhsT=HdT_sb[:, fh, c * 128:(c + 1) * 128],
                        rhs=WT_sb[:, fh, :],
                        start=(fh == 0), stop=(fh == 1))
                nc.vector.tensor_sub(
                    out=E_ext[:, c, 0:C], in0=xh_ps, in1=X_sb[:, c, :])
                s1_scr = psml.tile([128, C], f32, tag="sml")
                nc.vector.scalar_tensor_tensor(
                    out=s1_scr, in0=E_ext[:, c, 0:C], scalar=0.0,
                    in1=E_ext[:, c, 0:C], op0=Alu.bypass, op1=Alu.mult,
                    accum_out=acc_sb[:, 17 + c:18 + c])
                s2_scr = psml.tile([128, C], f32, tag="sml")
                nc.vector.scalar_tensor_tensor(
                    out=s2_scr, in0=xh_ps, scalar=0.0, in1=wd_bc,
                    op0=Alu.bypass, op1=Alu.mult,
                    accum_out=acc_sb[:, 25 + c:26 + c])
                nc.tensor.matmul(
                    out=grs_ps, lhsT=E_ext[:, c, :], rhs=Hd_sb[:, c, :],
                    start=(c == 0), stop=(c == NT - 1))
            grs_sb = consts.tile([C + 1, F], f32)
            nc.scalar.copy(out=grs_sb, in_=grs_ps)
            nc.sync.dma_start(out=grs_d[:, :], in_=grs_sb)

            # ---------- argmin + gathered loss terms per chunk ----------
            # dT holds -d, so max8/max_index give the (first-match) argmin.
            for c in range(NT):
                mx = psml.tile([128, 8], f32, tag="sm8")
                nc.vector.max(out=mx, in_=dT_all[:, c, :])
                mi = psml.tile([128, 8], mybir.dt.uint32, tag="sm8")
                nc.vector.max_index(out=mi, in_max=mx, in_values=dT_all[:, c, :])
                idx16 = psml.tile([128, 1], mybir.dt.uint16, tag="sm1")
                nc.vector.tensor_copy(out=idx16, in_=mi[:, 0:1])
                g16 = psml.tile([128, 16], f32, tag="sm16")
                nc.gpsimd.indirect_copy(
                    out=g16, data=G_sb[:, c, :], idxs=idx16,
                    i_know_ap_gather_is_preferred=True)
                s16 = psml.tile([128, 16], f32, tag="sm16")
                nc.vector.scalar_tensor_tensor(
                    out=s16, in0=g16, scalar=0.0, in1=diag16,
                    op0=Alu.bypass, op1=Alu.mult,
                    accum_out=acc_sb[:, c:c + 1])
                m16 = psml.tile([128, 16], f32, tag="sm16")
                nc.gpsimd.indirect_copy(
                    out=m16, data=msq_bc, idxs=idx16,
                    i_know_ap_gather_is_preferred=True)
                m16s = psml.tile([128, 16], f32, tag="sm16")
                nc.vector.scalar_tensor_tensor(
                    out=m16s, in0=m16, scalar=0.0, in1=diag16,
                    op0=Alu.bypass, op1=Alu.mult,
                    accum_out=acc_sb[:, 8 + c:9 + c])

            nc.sync.dma_start(out=acc_d[:, :], in_=acc_sb)

    nc.finalize()
    return nc


def _get_nc(reps=1):
    if reps not in _NC_CACHE:
        _NC_CACHE[reps] = _build_nc(reps)
    return _NC_CACHE[reps]


def _shard(inputs):
    X = np.ascontiguousarray(np.asarray(inputs["X"], dtype=np.float32))
    H = np.ascontiguousarray(np.asarray(inputs["H"], dtype=np.float32))
    M = np.ascontiguousarray(np.asarray(inputs["M"], dtype=np.float32))
    Hd = np.ascontiguousarray(np.asarray(inputs["Hdec"], dtype=np.float32))
    W = np.ascontiguousarray(np.asarray(inputs["W"], dtype=np.float32))
    wd = np.ascontiguousarray(
        np.asarray(inputs["w_d"], dtype=np.float32).reshape(1, C))
    in_maps = []
    for b in range(NCORES):
        in_maps.append({
            "H": np.ascontiguousarray(H[b]),
            "M": M,
            "X": np.ascontiguousarray(X[b]),
            "Hd": np.ascontiguousarray(Hd[b]),
            "W": W,
            "wd": wd,
        })
    return in_maps, wd


def _combine(results, wd):
    acc = np.stack([np.asarray(r["acc"]) for r in results]).astype(np.float64)
    grs = np.stack([np.asarray(r["grs"]) for r in results]).astype(np.float64)
    DOT = acc[:, :, 0:8].sum()
    MSQ = acc[:, :, 8:16].sum()
    HSQ = acc[:, :, 16].sum()
    S1 = acc[:, :, 17:25].sum()
    S2 = acc[:, :, 25:33].sum()
    GR = grs[:, 0:C, :].sum(axis=0)
    SV = grs[:, C, :].sum(axis=0)
    ntc = float(B * T * C)
    nh = float(B * D * T)
    loss_rec = S1 / ntc
    loss_d = -S2 / ntc
    loss_m = 2.0 * (HSQ - 2.0 * DOT + MSQ) / nh
    gr_norm = (2.0 / ntc) * np.linalg.norm(GR)
    gd_norm = (1.0 / ntc) * np.linalg.norm(wd.astype(np.float64)) \
        * np.linalg.norm(SV)
    lmbda = gr_norm / (gd_norm + GAMMA)
    out = loss_rec + ALPHA * loss_m + lmbda * loss_d
    return np.array(out, dtype=np.float32)


def run(inputs, trace=False):
    from concourse.bass_utils import run_bass_kernel_spmd
    nc = _get_nc()
    in_maps, wd = _shard(inputs)
    last_err = None
    for _attempt in range(3):
        try:
            res = run_bass_kernel_spmd(
                nc, in_maps, core_ids=list(range(NCORES)), trace=trace)
            return _combine(res.results, wd), res
        except Exception as e:  # transient axon-relay fetch failures
            last_err = e
    raise last_err


def kernel(**inputs) -> np.ndarray:
    out, _ = run(inputs, trace=False)
    return out
